# revision 9
# baseline (speedup 1.0000x reference)
"""Multi-head attention Trainium2 kernel.

Full inputs -> shard over 8 NeuronCores (batch x head-group) -> full output.

Per core c: batch b = c // 2, head-group hg = c % 2 (8 of 16 heads).
Column-shard Wq/Wk/Wv, row-shard Wo; each core computes a partial output
projection for its batch; host sums the two partials per batch and adds bo.

Layout strategy (per core):
  - host passes x^T = X[b].T  [D, S] so the d-contraction sits on partitions.
  - qT, kT computed as [d_head-stacked, S] (heads stacked 64+64 per 128-tile);
    v computed in natural [S, d] layout with a ones-column per head so the
    attn@V matmul also emits softmax row-sums for free.
  - scores^T tile [k,q] = kT_h^T-block  (K=64 matmuls, two heads packed into
    PE rows 0-63 / 64-127 run concurrently);
  - exp on ScalarE (scale=1/sqrt(dk) folded in) -> bf16; mask multiply on
    VectorE in bf16 (2x mode); attn@V accumulates ctx^T [65, q] in PSUM
    (row 64 = softmax denominators l).
  - normalize: reciprocal(l) -> DMA partition-broadcast -> VectorE multiply,
    writing stacked ctx^T tiles that feed the Wo projection directly.
  - all f32 matmuls run as float32r (full PE rate at N>=256).
"""

import os
import sys

for _p in ("/opt/trn_rl_repo", "/root/.axon_site/_ro/trn_rl_repo"):
    if os.path.isdir(_p) and _p not in sys.path:
        sys.path.insert(0, _p)

import numpy as np
import ml_dtypes

B, S, D, H = 4, 2048, 1024, 16
DK = 64
N_CORES = 8
HG = 2                  # head groups (cores per batch)
DH = D // HG            # 512: d_out per core
QC = 512                # q-chunk width (one PSUM bank)


def build_attention_nc(s=S, d=D, dh=DH, qc=QC):
    """Build the single-core Bass program (SPMD across 8 cores)."""
    import concourse.bass as bass  # noqa: F401
    import concourse.mybir as mybir
    import concourse.tile as tile
    from concourse import bacc

    f32 = mybir.dt.float32
    f32r = mybir.dt.float32r
    bf16 = mybir.dt.bfloat16
    EXPF = mybir.ActivationFunctionType.Exp

    n_h = dh // DK            # heads on this core (8)
    n_hp = n_h // 2           # head pairs (4)
    n_di = d // 128           # d_model 128-tiles (8)
    n_do = dh // 128          # d_out 128-tiles (4)
    n_kt = s // 128           # key 128-tiles (16)
    n_qc = s // qc            # q chunks (4)
    sc512 = min(512, s)       # projection free-dim chunk
    n_sc = s // sc512         # seq chunks for projections
    n_st = s // 128           # seq 128-tiles (16)
    qb = min(512, qc)         # matmul sub-chunk inside a q-chunk
    VA = n_h * 65             # v_aug width per seq-tile

    nc = bacc.Bacc(None, target_bir_lowering=False)

    xqT = nc.dram_tensor("xqT", [d, s], f32r, kind="ExternalInput")
    xkT = nc.dram_tensor("xkT", [d, s], f32r, kind="ExternalInput")
    xvT = nc.dram_tensor("xvT", [d, s], f32r, kind="ExternalInput")
    maskT = nc.dram_tensor("maskT", [s, s], bf16, kind="ExternalInput")
    wq = nc.dram_tensor("wq", [d, dh], f32r, kind="ExternalInput")
    wk = nc.dram_tensor("wk", [d, dh], f32r, kind="ExternalInput")
    wv = nc.dram_tensor("wv", [d, dh], f32r, kind="ExternalInput")
    wo = nc.dram_tensor("wo", [dh, d], f32r, kind="ExternalInput")
    bq = nc.dram_tensor("bq", [1, dh], f32r, kind="ExternalInput")
    bk = nc.dram_tensor("bk", [1, dh], f32r, kind="ExternalInput")
    bv = nc.dram_tensor("bv", [1, dh], f32r, kind="ExternalInput")
    ones_d = nc.dram_tensor("ones_d", [1, 512], f32r, kind="ExternalInput")
    oT = nc.dram_tensor("oT", [d, s], f32, kind="ExternalOutput")

    with tile.TileContext(nc) as tc:
        with (
            tc.tile_pool(name="qk", bufs=2 * n_do) as qk_pool,
            tc.tile_pool(name="va", bufs=n_st) as va_pool,
            tc.tile_pool(name="const", bufs=4) as const_pool,
        ):
            ones = const_pool.tile([1, 512], f32r, tag="ones", name="ones")
            nc.sync.dma_start(ones[:, :], ones_d[:, :])
            bq_sb = const_pool.tile([1, dh], f32r, tag="bias", name="bq_sb")
            bk_sb = const_pool.tile([1, dh], f32r, tag="bias", name="bk_sb")
            bv_sb = const_pool.tile([1, dh], f32r, tag="bias", name="bv_sb")
            nc.sync.dma_start(bq_sb[:, :], bq[:, :])
            nc.sync.dma_start(bk_sb[:, :], bk[:, :])
            nc.sync.dma_start(bv_sb[:, :], bv[:, :])

            # ---------------- phase 1a: V projection (natural layout) -------
            va_tiles = []
            with (
                tc.tile_pool(name="stage", bufs=n_di) as stage_pool,
                tc.tile_pool(name="w", bufs=n_di) as w_pool,
                tc.tile_pool(name="p1psum", bufs=3, space="PSUM") as p1_psum,
                tc.tile_pool(name="p1out", bufs=4) as p1out_pool,
            ):
                wv_tiles = []
                for di in range(n_di):
                    wt = w_pool.tile([128, dh], f32r, tag="w", name="wt")
                    nc.sync.dma_start(wt[:, :], wv[di * 128:(di + 1) * 128, :])
                    wv_tiles.append(wt)
                xv_tiles = []
                for di in range(n_di):
                    xt = stage_pool.tile([128, s], f32r, tag="x", name="xt")
                    nc.sync.dma_start(xt[:, :], xvT[di * 128:(di + 1) * 128, :])
                    xv_tiles.append(xt)

                for st in range(n_st):
                    ps = p1_psum.tile([128, dh], f32, tag="ps", name="ps")
                    for di in range(n_di):
                        nc.tensor.matmul(
                            ps[:, :],
                            xv_tiles[di][:, st * 128:(st + 1) * 128],
                            wv_tiles[di][:, :],
                            start=(di == 0), stop=False,
                        )
                    # + bias (rank-1: ones^T x bv)
                    nc.tensor.matmul(
                        ps[:, :], ones[:, 0:128], bv_sb[:, :],
                        start=False, stop=True,
                    )
                    va = va_pool.tile([128, VA], bf16, tag="va", name="va")
                    va3 = va.rearrange("p (h x) -> p h x", x=65)
                    nc.vector.memset(va3[:, :, 64:65], 1.0)
                    nc.scalar.copy(
                        va3[:, :, 0:64],
                        ps[:, :].rearrange("p (h x) -> p h x", x=64),
                    )
                    va_tiles.append(va)

                # ------------- phase 1b: K^T then Q^T projections ----------
                qkt = {}
                for name, wdram, bsb, xdram in (
                    ("k", wk, bk_sb, xkT), ("q", wq, bq_sb, xqT),
                ):
                    wts = []
                    for di in range(n_di):
                        wt = w_pool.tile([128, dh], f32r, tag="w", name="wt")
                        nc.sync.dma_start(wt[:, :], wdram[di * 128:(di + 1) * 128, :])
                        wts.append(wt)
                    xts = []
                    for di in range(n_di):
                        xt = stage_pool.tile([128, s], f32r, tag="x", name="xt")
                        nc.sync.dma_start(xt[:, :], xdram[di * 128:(di + 1) * 128, :])
                        xts.append(xt)
                    outs = []
                    for do in range(n_do):
                        ot = qk_pool.tile([128, s], f32r, tag="qk", name="qkt")
                        outs.append(ot)
                        for sc in range(n_sc):
                            ps = p1_psum.tile([128, sc512], f32, tag="ps", name="ps")
                            for di in range(n_di):
                                nc.tensor.matmul(
                                    ps[:, :],
                                    wts[di][:, do * 128:(do + 1) * 128],
                                    xts[di][:, sc * sc512:(sc + 1) * sc512],
                                    start=(di == 0), stop=False,
                                )
                            nc.tensor.matmul(
                                ps[:, :],
                                bsb[:, do * 128:(do + 1) * 128],
                                ones[:, 0:sc512],
                                start=False, stop=True,
                            )
                            nc.scalar.copy(ot[:, sc * sc512:(sc + 1) * sc512], ps[:, :])
                    qkt[name] = outs

            kT, qT = qkt["k"], qkt["q"]

            # ---------------- phase 2: attention -----------------------------
            scale = 1.0 / np.sqrt(np.float32(DK))
            with (
                tc.tile_pool(name="ctxT", bufs=n_do) as ctxT_pool,
                tc.tile_pool(name="mask", bufs=n_kt + 2) as mask_pool,
                tc.tile_pool(name="e", bufs=4) as e_pool,
                tc.tile_pool(name="p", bufs=4) as p_pool,
                tc.tile_pool(name="nrm", bufs=6) as nrm_pool,
                tc.tile_pool(name="spsum", bufs=4, space="PSUM") as s_psum,
                tc.tile_pool(name="cpsum", bufs=2, space="PSUM") as c_psum,
                tc.tile_pool(name="p3psum", bufs=2, space="PSUM") as p3_psum,
            ):
                ctxT = [ctxT_pool.tile([128, s], f32r, tag="ctxT",
                                       name=f"ctxT{t}")
                        for t in range(n_do)]
                nqb = qc // qb  # sub-chunk matmuls per q-chunk
                for qcb in range(n_qc):
                    mts = []
                    for kt in range(n_kt):
                        mt = mask_pool.tile([128, qc], bf16, tag="m", name="mt")
                        nc.sync.dma_start(
                            mt[:, :],
                            maskT[kt * 128:(kt + 1) * 128, qcb * qc:(qcb + 1) * qc],
                        )
                        mts.append(mt)
                    for hp in range(n_hp):
                        tl = hp               # qT/kT 128-tile holding heads 2hp, 2hp+1
                        cps = []
                        for hh in range(2):
                            cps.append(c_psum.tile([65, qc], f32, tag="c", name="cp"))
                        for kt in range(n_kt):
                            pts = []
                            for hh in range(2):
                                h = hp * 2 + hh
                                lo = hh * 64
                                sp = s_psum.tile([128, qc], f32, tag="s", name="sp")
                                for qi in range(nqb):
                                    nc.tensor.matmul(
                                        sp[:, qi * qb:(qi + 1) * qb],
                                        kT[tl][lo:lo + 64, kt * 128:(kt + 1) * 128],
                                        qT[tl][lo:lo + 64,
                                                 qcb * qc + qi * qb:
                                                 qcb * qc + (qi + 1) * qb],
                                        start=True, stop=True,
                                    )
                                et = e_pool.tile([128, qc], bf16, tag="e", name="et")
                                nc.scalar.activation(et[:, :], sp[:, :], EXPF,
                                                     scale=float(scale))
                                pt = p_pool.tile([128, qc], bf16, tag="p", name="pt")
                                nc.vector.tensor_mul(pt[:, :], et[:, :], mts[kt][:, :])
                                pts.append(pt)
                            for hh in range(2):
                                h = hp * 2 + hh
                                for qi in range(nqb):
                                    nc.tensor.matmul(
                                        cps[hh][:, qi * qb:(qi + 1) * qb],
                                        va_tiles[kt][:, h * 65:(h + 1) * 65],
                                        pts[hh][:, qi * qb:(qi + 1) * qb],
                                        start=(kt == 0), stop=(kt == n_kt - 1),
                                        skip_group_check=True,
                                    )
                        # normalize: ctx[0:64] / ctx[64] -> stacked ctxT
                        for hh in range(2):
                            rt = nrm_pool.tile([1, qc], f32, tag="r", name="rt")
                            nc.vector.reciprocal(rt[:, :], cps[hh][64:65, :])
                            bc = nrm_pool.tile([64, qc], f32, tag="bc", name="bc")
                            nc.gpsimd.partition_broadcast(bc[:, :], rt[0:1, :], 64)
                            nc.vector.tensor_mul(
                                ctxT[hp][hh * 64:hh * 64 + 64,
                                         qcb * qc:(qcb + 1) * qc],
                                cps[hh][0:64, :], bc[:, :])

                # ----------- phase 3: output projection ----------------------
                with tc.tile_pool(name="wo", bufs=n_do) as wo_pool:
                    wo_tiles = []
                    for t in range(n_do):
                        wt = wo_pool.tile([128, d], f32r, tag="wo", name="wot")
                        nc.sync.dma_start(wt[:, :], wo[t * 128:(t + 1) * 128, :])
                        wo_tiles.append(wt)
                    for dm in range(n_di):
                        for sc in range(n_sc):
                            ps = p3_psum.tile([128, sc512], f32, tag="o", name="ops")
                            for t in range(n_do):
                                nc.tensor.matmul(
                                    ps[:, :],
                                    wo_tiles[t][:, dm * 128:(dm + 1) * 128],
                                    ctxT[t][:, sc * sc512:(sc + 1) * sc512],
                                    start=(t == 0), stop=(t == n_do - 1),
                                )
                            osb = wo_pool.tile([128, sc512], f32,
                                               tag="osb", bufs=3, name="osb")
                            nc.scalar.copy(osb[:, :], ps[:, :])
                            nc.sync.dma_start(
                                oT[dm * 128:(dm + 1) * 128,
                                   sc * sc512:(sc + 1) * sc512],
                                osb[:, :],
                            )

    nc.compile()
    return nc


def to_f32r(a):
    """Round fp32 to the PE's fp32r storage format (RNE to 11-bit mantissa)."""
    u = np.ascontiguousarray(a, np.float32).view(np.uint32).copy()
    lsb = (u >> np.uint32(12)) & np.uint32(1)
    u += np.uint32(0x7FF) + lsb
    u &= np.uint32(0xFFFFF000)
    return u.view(np.float32)


def make_in_maps(Q, K, V, mask, Wq, bq, Wk, bk, Wv, bv, Wo):
    Q = np.asarray(Q, np.float32)
    K = np.asarray(K, np.float32)
    V = np.asarray(V, np.float32)
    mask = np.asarray(mask)
    in_maps = []
    for c in range(N_CORES):
        b, hg = c // HG, c % HG
        cs = slice(hg * DH, (hg + 1) * DH)
        in_maps.append({
            "xqT": to_f32r(Q[b].T),
            "xkT": to_f32r(K[b].T),
            "xvT": to_f32r(V[b].T),
            "maskT": np.ascontiguousarray(mask[b, 0].T).astype(ml_dtypes.bfloat16),
            "wq": to_f32r(np.asarray(Wq, np.float32)[:, cs]),
            "wk": to_f32r(np.asarray(Wk, np.float32)[:, cs]),
            "wv": to_f32r(np.asarray(Wv, np.float32)[:, cs]),
            "wo": to_f32r(np.asarray(Wo, np.float32)[cs, :]),
            "bq": to_f32r(np.asarray(bq, np.float32)[cs].reshape(1, DH)),
            "bk": to_f32r(np.asarray(bk, np.float32)[cs].reshape(1, DH)),
            "bv": to_f32r(np.asarray(bv, np.float32)[cs].reshape(1, DH)),
            "ones_d": np.ones((1, 512), np.float32),
        })
    return in_maps


def combine_outputs(results, bo):
    out = np.empty((B, S, D), np.float32)
    for b in range(B):
        out[b] = (results[HG * b]["oT"] + results[HG * b + 1]["oT"]).T
    out += np.asarray(bo, np.float32)
    return out


def kernel(Q, K, V, mask, Wq, bq, Wk, bk, Wv, bv, Wo, bo):
    from concourse.bass_utils import run_bass_kernel_spmd

    in_maps = make_in_maps(Q, K, V, mask, Wq, bq, Wk, bk, Wv, bv, Wo)
    nc = build_attention_nc()
    res = run_bass_kernel_spmd(nc, in_maps, core_ids=list(range(N_CORES)))
    return combine_outputs(res.results, bo)
